# revision 23
# baseline (speedup 1.0000x reference)
"""AFNO spectral attention kernel for 8 TRN2 NeuronCores.

Math: the reference's rfft2 -> truncate -> per-block mode mix -> irfft2
collapses to a per-block real 224x224 matrix A_b applied along the W axis
(the H-direction FFT commutes with the mode mixing and cancels), plus a
bias-driven constant on the n_h==0 spatial rows. The residual+projection
out = Xs + Xs @ (rescale*proj_w.T) runs as a mixed-precision PSUM
accumulation:
  - the heavy projection term uses fp8(e4m3) DoubleRow matmuls (K=192 per
    instruction, 2x PE rate); W' is pre-scaled by 128 so its ~0.002-scale
    entries land in fp8 normal range,
  - the identity term re-adds Xs from bf16 via scaled-identity matmuls
    (rhs = 128*I, exact in bf16) into the same PSUM bank,
  - the PSUM->SBUF copy descales by 1/128.
Only ~5.5% of the output magnitude flows through fp8, so the added error
is ~0.3% RMS.

Sharding: 100352 tokens = 8 cores x 12544 (56 complete image rows per
core, batch boundary lands exactly on the core-4 boundary). No
collectives needed.

Layout: stage-1 leaves Xs in per-block [96-partition, block, token] form
consumed directly by stage-2 (fp8 lhsT pairs two 96-channel blocks per
DoubleRow matmul), which removes the 96->128 channel repack -- the DVE
32-wide shuffle copies that dominated the previous version (86% DVE
busy). DVE now only runs the bf16->fp8 cast of Xs. Output is written
bf16 (error budget allows) and upcast on the host, halving store
traffic.
"""

import numpy as np
import ml_dtypes

import concourse.bass as bass
import concourse.mybir as mybir
import concourse.tile as tile
from concourse.bass_utils import run_bass_kernel_spmd

B, Hh, Ww, C = 2, 224, 224, 768
NB, BS, M = 8, 96, 96
NMODES = Ww // 2 + 1  # 113
N_CORES = 8
TOK = B * Hh * Ww  # 100352 total tokens
TOK_CORE = TOK // N_CORES  # 12544
ROWS_CORE = TOK_CORE // Ww  # 56 image rows per core
RG = 4  # image rows per group
GROUPS = ROWS_CORE // RG  # 14
TG = RG * Ww  # tokens per group = 896
TCH = TG // 128  # t-chunks of 128 per group = 7
A_COLS = 2 * NB * Ww  # 3584
I_COLS = BS  # identity block
WC_COLS = A_COLS + I_COLS  # 3680
NPAIR = NB // 2  # 4 block pairs for fp8 DoubleRow
HALF = NPAIR * BS  # 384 output cols per psum half
FSCALE = 128.0  # fp8 weight pre-scale (descaled on PSUM copy-out)

BF16 = ml_dtypes.bfloat16
FP8 = ml_dtypes.float8_e4m3

_CACHE = {}


def _build_amat(block_weights, gates):
    """Per-block real [224, 224] spatial-W operator."""
    g = 1.0 / (1.0 + np.exp(-gates.astype(np.float64)))
    F = np.fft.rfft(np.eye(Ww), axis=1, norm="ortho")  # (224, 113)
    A = np.zeros((NB, Ww, Ww), np.float64)
    for b in range(NB):
        T = np.zeros((NMODES, NMODES), np.complex128)
        T[:M, :M] = g[b] * block_weights[b].astype(np.float64)
        for k in range(M, NMODES):
            T[k, k] = 1.0
        A[b] = np.fft.irfft(F @ T, n=Ww, axis=1, norm="ortho")
    return A, g


def _bias_const_rows(block_bias, g):
    """Constant added to spatial rows n_h == 0, per block: (NB, 224)."""
    rows = np.zeros((NB, Ww), np.float64)
    for b in range(NB):
        z = np.zeros(NMODES, np.complex128)
        z[:M] = g[b] * block_bias[b].astype(np.float64) * (1.0 + 1.0j)
        rows[b] = np.sqrt(Hh) * np.fft.irfft(z, n=Ww, norm="ortho")
    return rows


def _pack_weights(A, Wp):
    """bf16 [128, 3680]: A chunks (k=0 rows 0:128, k=1 rows 128:224) then
    the scaled 96x96 identity; fp8 [96, 4, 2, 768]: W'*FSCALE by block
    pair."""
    wc = np.zeros((128, WC_COLS), np.float32)
    for k in range(2):
        for b in range(NB):
            blk = A[b, k * 128 : min((k + 1) * 128, Ww), :]  # (128|96, 224)
            wc[: blk.shape[0], k * NB * Ww + b * Ww : k * NB * Ww + (b + 1) * Ww] = blk
    # W' + I: the a-slot identity rides the fp8 weight diagonal (128 is
    # fp8-exact; the ~0.25-max scaled W' diagonal rounds away under the
    # ulp-16 step at 128 -- a ~0.2% RMS loss, within budget).
    WpI = Wp + np.eye(C)
    # The repack DMA pairs src rows (p-major over [96p, 8b]) with dst rows
    # (q-major over [128q, 6k]) in flat order, so dst cell (q, k) holds
    # channel 96*((6q+k)%8) + (6q+k)//8. Pack W' rows in that order.
    w8r = np.zeros((128, 3, 2, C), np.float32)
    for q in range(128):
        for k in range(6):
            d = q * 6 + k
            c = 96 * (d % 8) + d // 8
            w8r[q, k // 2, k % 2, :] = WpI[c, :] * FSCALE
    # paired b-residual identity: [96, 2, 192] = 128*[I|0], 128*[0|I]
    i8 = np.zeros((BS, 2, 2 * BS), np.float32)
    i8[:, 0, 0:BS] = np.eye(BS) * FSCALE
    i8[:, 1, BS : 2 * BS] = np.eye(BS) * FSCALE
    return wc.astype(BF16), w8r.astype(FP8), i8.astype(FP8)


def _elide_redundant_waits(nc):
    """Drop per-instruction semaphore waits already implied by the
    instruction's other waits (transitively, via the wait chains of the
    instructions that perform the increments). Tile's sem assignment is
    per-proc minimal but not transitively minimal across procs, and
    walrus's per-instruction sync-command budget is tiny (matmul fits
    only one wait + one update)."""
    fn = nc.m.functions[0]
    implied = {}  # sem name -> [state dict after k-th increment]
    engine_state = {}  # engine -> folded state of prior instructions' waits

    def state_of(sem, v):
        lst = implied.get(sem)
        if not lst or v <= 0:
            return {}
        return lst[min(v, len(lst)) - 1]

    def fold(dst, src):
        for s, v in src.items():
            if dst.get(s, 0) < v:
                dst[s] = v

    own_updates = {}  # engine -> {sem: count of updates emitted by this engine}
    sem_updaters = {}  # sem -> set of (engine, is_dma) that updated it
    for blk in fn.blocks:
        for inst in blk.instructions:
            si = inst.sync_info
            eng = getattr(inst, "engine", None)
            is_dma = "DMA" in type(inst).__name__
            waits = list(si.on_wait or []) if si else []
            # prune waits on this engine's own completion sem: a compute
            # engine executes serially, so all its prior updates are done
            # by the time this instruction runs. Only valid when every
            # updater of the sem so far was this engine's synchronous
            # (non-DMA) instructions.
            if eng is not None and not is_dma and waits:
                keep0 = []
                for w in waits:
                    ups = sem_updaters.get(w.ant_name)
                    if (
                        w.wait_value is not None
                        and ups is not None
                        and ups == {(eng, False)}
                        and own_updates.get(eng, {}).get(w.ant_name, 0)
                        >= w.wait_value
                    ):
                        continue
                    keep0.append(w)
                if len(keep0) != len(waits):
                    si.on_wait = keep0
                    waits = keep0
            my = dict(engine_state.get(eng, {}))
            for w in waits:
                if w.wait_value is None:
                    continue
                fold(my, {w.ant_name: w.wait_value})
                fold(my, state_of(w.ant_name, w.wait_value))
            if len(waits) > 1 and all(w.wait_value is not None for w in waits):
                keep = []
                for w in waits:
                    others = dict(engine_state.get(eng, {}))
                    for w2 in waits:
                        if w2 is w:
                            continue
                        fold(others, {w2.ant_name: w2.wait_value})
                        fold(others, state_of(w2.ant_name, w2.wait_value))
                    if others.get(w.ant_name, -1) >= w.wait_value:
                        continue
                    keep.append(w)
                if len(keep) != len(waits):
                    si.on_wait = keep
            if eng is not None:
                engine_state[eng] = my
            for u in (si.on_update or []) if si else []:
                nm = u.ant_name
                lst = implied.setdefault(nm, [])
                prev = dict(lst[-1]) if lst else {}
                fold(prev, my)
                n = u.update_value or 1
                prev[nm] = len(lst) + n
                for _ in range(int(n)):
                    lst.append(prev)
                if eng is not None:
                    eu = own_updates.setdefault(eng, {})
                    eu[nm] = eu.get(nm, 0) + int(n)
                sem_updaters.setdefault(nm, set()).add((eng, is_dma))


def _build_nc():
    nc = bass.Bass("TRN2", target_bir_lowering=False)
    x_ext = nc.declare_dram_parameter(
        "x", [ROWS_CORE, Ww, C], mybir.dt.bfloat16, isOutput=False
    )
    w_ext = nc.declare_dram_parameter(
        "wconst", [128, WC_COLS], mybir.dt.bfloat16, isOutput=False
    )
    w8_ext = nc.declare_dram_parameter(
        "wconst8", [128, 3, 2, C], mybir.dt.float8e4, isOutput=False
    )
    i8_ext = nc.declare_dram_parameter(
        "wconst8i", [BS, 2, 2 * BS], mybir.dt.float8e4, isOutput=False
    )
    out_ext = nc.declare_dram_parameter(
        "out", [TOK_CORE, C], mybir.dt.bfloat16, isOutput=True
    )

    def a_sl(k, b):
        return slice(k * NB * Ww + b * Ww, k * NB * Ww + (b + 1) * Ww)

    with tile.TileContext(nc) as tc:
        with (
            tc.tile_pool(name="const", bufs=1) as const_pool,
            tc.tile_pool(name="xin", bufs=2) as x_pool,
            tc.tile_pool(name="xs", bufs=3) as xs_pool,
            tc.tile_pool(name="gps", bufs=2, space="PSUM") as g_psum,
            tc.tile_pool(name="ops", bufs=2, space="PSUM") as o_psum,
            tc.tile_pool(name="osb", bufs=6) as out_pool,
        ):
            wc = const_pool.tile([128, WC_COLS], mybir.dt.bfloat16)
            w8 = const_pool.tile([128, 3, 2, C], mybir.dt.float8e4)
            i8 = const_pool.tile([BS, 2, 2 * BS], mybir.dt.float8e4)
            nc.sync.dma_start(wc[:, :], w_ext[:, :])
            nc.sync.dma_start(w8[:, :, :, :], w8_ext[:, :, :, :])
            nc.sync.dma_start(i8[:, :, :], i8_ext[:, :, :])

            def load_x(g):
                xt = x_pool.tile([128, RG, 2, C], mybir.dt.bfloat16, tag="xin")
                src = x_ext[g * RG : (g + 1) * RG, :, :]
                nc.gpsimd.dma_start(
                    xt[:, :, 0, :], src[:, 0:128, :].rearrange("r p c -> p r c")
                )
                nc.gpsimd.dma_start(
                    xt[0:96, :, 1, :], src[:, 128:224, :].rearrange("r p c -> p r c")
                )
                return xt

            def stage1(g, xt):
                """returns (xs8, xs8r): xs8 [96, NB, 2, TG] with slot 0 =
                fp8(Xs), slot 1 = fp8 residual (Xs - slot0); xs8r is the
                a-slots repacked 128-dense by one SBUF->SBUF DMA (channel
                permutation absorbed into the host w8 packing)."""
                xs8 = xs_pool.tile([BS, NB, 2, TG], mybir.dt.float8e4, tag="xs8")
                xs8r = xs_pool.tile([128, 6, TG], mybir.dt.float8e4, tag="xs8r")
                for b in range(NB):
                    # [96, 2 banks, 512]: rows r at (r//2, (r%2)*224)
                    pg = g_psum.tile([BS, 2, 512], mybir.dt.float32, tag="gps")
                    for r in range(RG):
                        o = (r % 2) * Ww
                        for k in range(2):
                            kp = 128 if k == 0 else 96
                            nc.tensor.matmul(
                                pg[:, r // 2, o : o + Ww],
                                lhsT=xt[0:kp, r, k, b * BS : (b + 1) * BS],
                                rhs=wc[0:kp, a_sl(k, b)],
                                start=(k == 0),
                                stop=(k == 1),
                            )
                    # last 5 blocks' a-copies ride DVE to balance ACT; DVE
                    # blocks last so the repack's DVE wait transitively
                    # implies the ACT a-copies via the pg-pool WAR chain.
                    if b < 3:
                        nc.scalar.copy(xs8[:, b, 0, :], pg[:, :, 0 : 2 * Ww])
                    else:
                        nc.vector.tensor_copy(xs8[:, b, 0, :], pg[:, :, 0 : 2 * Ww])
                    nc.vector.tensor_sub(
                        xs8[:, b, 1, :], pg[:, :, 0 : 2 * Ww], xs8[:, b, 0, :]
                    )
                # two token-halves: parallel DMA engines, and each stage-2
                # chunk's ldweights waits on only one completion sem.
                nc.sync.dma_start(xs8r[:, :, 0:512], xs8[0:BS, :, 0, 0:512])
                nc.sync.dma_start(xs8r[:, :, 512:TG], xs8[0:BS, :, 0, 512:TG])
                return xs8, xs8r

            def stage2(g, xs):
                xs8, xs8r = xs
                ots = []
                for j in range(TCH):
                    # one 2-bank psum tile; halves accumulate independently
                    po = o_psum.tile([128, 2, 512], mybir.dt.float32, tag="po")
                    ts = slice(j * 128, (j + 1) * 128)
                    for p in range(3):
                        lhsT = xs8r[:, 2 * p : 2 * p + 2, ts]
                        for h in range(2):
                            nc.tensor.matmul(
                                po[:, h, 0:HALF],
                                lhsT=lhsT,
                                rhs=w8[:, p, :, h * HALF : (h + 1) * HALF],
                                start=(p == 0), stop=False,
                                perf_mode=mybir.MatmulPerfMode.DoubleRow,
                                skip_group_check=True,
                            )
                    # b-residual identity, two blocks per matmul
                    for p in range(NPAIR):
                        nc.tensor.matmul(
                            po[:, p // 2, (p % 2) * 2 * BS : (p % 2 + 1) * 2 * BS],
                            lhsT=xs8[:, 2 * p : 2 * p + 2, 1, ts],
                            rhs=i8[:, :, :],
                            start=False, stop=(p % 2 == 1),
                            perf_mode=mybir.MatmulPerfMode.DoubleRow,
                            skip_group_check=True,
                        )
                    # one spare column: the 1-element gate copy absorbs
                    # ot's slot-free (DMA) wait on the ACT queue so the
                    # real copy only carries the PE wait (walrus allows
                    # one wait per Activation), without WAW overlap.
                    ot = out_pool.tile([128, C + 1], mybir.dt.bfloat16, tag="osb")
                    nc.scalar.copy(ot[0:1, C : C + 1], wc[0:1, 0:1])
                    nc.scalar.mul(ot[:, 0:C], po[:, :, 0:HALF], 1.0 / FSCALE)
                    t0 = g * TG + j * 128
                    nc.sync.dma_start(out_ext[t0 : t0 + 128, :], ot[:, 0:C])
                    ots.append(ot)
                return ots

            # software pipeline: stage2(g-2) BEFORE stage1(g) in program
            # order -- two stages of slack hide the xs8r repack DMA, and
            # the out-copies precede group g's a-copies on the ACT queue so
            # the po-pool WAR doesn't couple PE to the a-copy backlog.
            xs_hist = []
            last_ots = None
            xt = load_x(0)
            for g in range(GROUPS + 2):
                if g >= 2:
                    last_ots = stage2(g - 2, xs_hist[g - 2])
                if g < GROUPS:
                    xs_hist.append(stage1(g, xt))
                    if g + 1 < GROUPS:
                        xt = load_x(g + 1)
            # tail joins: tiny ACT writes into the last out tiles make the
            # ACT queue observe the final out-DMA completions (WAR), so the
            # kernel-tail Drain's DMA-lane waits become implied and are
            # elided (walrus allows only one wait on Drain).
            for ot in last_ots[-6:]:
                nc.scalar.copy(ot[0:1, 0:1], wc[0:1, 0:1])

    _elide_redundant_waits(nc)
    return nc


def kernel(x, block_weights, block_bias, gates, proj_w, proj_b, rescale):
    x = np.asarray(x)
    A, g = _build_amat(np.asarray(block_weights), np.asarray(gates))
    Wp = float(rescale) * np.asarray(proj_w, np.float64).T  # [c, o], no identity
    wc_host, w8_host, i8_host = _pack_weights(A, Wp)

    # shard 56 image rows per core
    x_rows = np.ascontiguousarray(x.reshape(TOK // Ww, Ww, C).astype(BF16))

    if "nc" not in _CACHE:
        _CACHE["nc"] = _build_nc()
    nc = _CACHE["nc"]

    in_maps = []
    for i in range(N_CORES):
        in_maps.append(
            {
                "x": x_rows[i * ROWS_CORE : (i + 1) * ROWS_CORE],
                "wconst": wc_host,
                "wconst8": w8_host,
                "wconst8i": i8_host,
            }
        )
    res = run_bass_kernel_spmd(
        nc,
        in_maps,
        core_ids=list(range(N_CORES)),
        trace=bool(_CACHE.get("trace", False)),
        **_CACHE.get("trace_kwargs", {}),
    )
    _CACHE["last_results"] = res
    out = np.concatenate([r["out"] for r in res.results], axis=0)
    out = out.reshape(B, Hh * Ww, C).astype(np.float32)

    # host-side constant corrections (zero for the reference inputs)
    bb = np.asarray(block_bias)
    pb = np.asarray(proj_b)
    if np.any(bb) or np.any(pb):
        P = Wp + np.eye(C)
        const = np.zeros((Hh * Ww, C), np.float64)
        if np.any(bb):
            rows = _bias_const_rows(bb, g)  # (NB, 224)
            cr = np.zeros((Ww, C), np.float64)
            for b in range(NB):
                cr[:, b * BS : (b + 1) * BS] = rows[b][:, None]
            # affects tokens with n_h == 0: tokens 0..223 of each batch image
            const[0:Ww, :] = cr @ P  # x_const goes through out = x_const @ P
        add = const[None, :, :] + float(rescale) * pb.astype(np.float64)[None, None, :]
        out = (out.astype(np.float64) + add).astype(np.float32)
    return out


# revision 24
# speedup vs baseline: 1.1483x; 1.1483x over previous
"""AFNO spectral attention kernel for 8 TRN2 NeuronCores.

Math: the reference's rfft2 -> truncate -> per-block mode mix -> irfft2
collapses to a per-block real 224x224 matrix A_b applied along the W axis
(the H-direction FFT commutes with the mode mixing and cancels), plus a
bias-driven constant on the n_h==0 spatial rows. The residual+projection
out = Xs + Xs @ (rescale*proj_w.T) runs as a mixed-precision PSUM
accumulation:
  - the heavy projection term uses fp8(e4m3) DoubleRow matmuls (K=192 per
    instruction, 2x PE rate); W' is pre-scaled by 128 so its ~0.002-scale
    entries land in fp8 normal range,
  - the identity term re-adds Xs from bf16 via scaled-identity matmuls
    (rhs = 128*I, exact in bf16) into the same PSUM bank,
  - the PSUM->SBUF copy descales by 1/128.
Only ~5.5% of the output magnitude flows through fp8, so the added error
is ~0.3% RMS.

Sharding: 100352 tokens = 8 cores x 12544 (56 complete image rows per
core, batch boundary lands exactly on the core-4 boundary). No
collectives needed.

Layout: stage-1 leaves Xs in per-block [96-partition, block, token] form
consumed directly by stage-2 (fp8 lhsT pairs two 96-channel blocks per
DoubleRow matmul), which removes the 96->128 channel repack -- the DVE
32-wide shuffle copies that dominated the previous version (86% DVE
busy). DVE now only runs the bf16->fp8 cast of Xs. Output is written
bf16 (error budget allows) and upcast on the host, halving store
traffic.
"""

import numpy as np
import ml_dtypes

import concourse.bass as bass
import concourse.mybir as mybir
import concourse.tile as tile
from concourse.bass_utils import run_bass_kernel_spmd

B, Hh, Ww, C = 2, 224, 224, 768
NB, BS, M = 8, 96, 96
NMODES = Ww // 2 + 1  # 113
N_CORES = 8
TOK = B * Hh * Ww  # 100352 total tokens
TOK_CORE = TOK // N_CORES  # 12544
ROWS_CORE = TOK_CORE // Ww  # 56 image rows per core
RG = 4  # image rows per group
GROUPS = ROWS_CORE // RG  # 14
TG = RG * Ww  # tokens per group = 896
TCH = TG // 128  # t-chunks of 128 per group = 7
A_COLS = 2 * NB * Ww  # 3584
I_COLS = BS  # identity block
WC_COLS = A_COLS + I_COLS  # 3680
NPAIR = NB // 2  # 4 block pairs for fp8 DoubleRow
HALF = NPAIR * BS  # 384 output cols per psum half
FSCALE = 128.0  # fp8 weight pre-scale (descaled on PSUM copy-out)

BF16 = ml_dtypes.bfloat16
FP8 = ml_dtypes.float8_e4m3

_CACHE = {}


def _build_amat(block_weights, gates):
    """Per-block real [224, 224] spatial-W operator."""
    g = 1.0 / (1.0 + np.exp(-gates.astype(np.float64)))
    F = np.fft.rfft(np.eye(Ww), axis=1, norm="ortho")  # (224, 113)
    A = np.zeros((NB, Ww, Ww), np.float64)
    for b in range(NB):
        T = np.zeros((NMODES, NMODES), np.complex128)
        T[:M, :M] = g[b] * block_weights[b].astype(np.float64)
        for k in range(M, NMODES):
            T[k, k] = 1.0
        A[b] = np.fft.irfft(F @ T, n=Ww, axis=1, norm="ortho")
    return A, g


def _bias_const_rows(block_bias, g):
    """Constant added to spatial rows n_h == 0, per block: (NB, 224)."""
    rows = np.zeros((NB, Ww), np.float64)
    for b in range(NB):
        z = np.zeros(NMODES, np.complex128)
        z[:M] = g[b] * block_bias[b].astype(np.float64) * (1.0 + 1.0j)
        rows[b] = np.sqrt(Hh) * np.fft.irfft(z, n=Ww, norm="ortho")
    return rows


def _pack_weights(A, Wp):
    """bf16 [128, 3680]: A chunks (k=0 rows 0:128, k=1 rows 128:224) then
    the scaled 96x96 identity; fp8 [96, 4, 2, 768]: W'*FSCALE by block
    pair."""
    wc = np.zeros((128, WC_COLS), np.float32)
    for k in range(2):
        for b in range(NB):
            blk = A[b, k * 128 : min((k + 1) * 128, Ww), :]  # (128|96, 224)
            wc[: blk.shape[0], k * NB * Ww + b * Ww : k * NB * Ww + (b + 1) * Ww] = blk
    # W' + I: the a-slot identity rides the fp8 weight diagonal (128 is
    # fp8-exact; the ~0.25-max scaled W' diagonal rounds away under the
    # ulp-16 step at 128 -- a ~0.2% RMS loss, within budget).
    WpI = Wp + np.eye(C)
    # The repack DMA pairs src rows (p-major over [96p, 8b]) with dst rows
    # (q-major over [128q, 6k]) in flat order, so dst cell (q, k) holds
    # channel 96*((6q+k)%8) + (6q+k)//8. Pack W' rows in that order.
    w8r = np.zeros((128, 3, 2, C), np.float32)
    for q in range(128):
        for k in range(6):
            d = q * 6 + k
            c = 96 * (d % 8) + d // 8
            w8r[q, k // 2, k % 2, :] = WpI[c, :] * FSCALE
    # paired b-residual identity: [96, 2, 192] = 128*[I|0], 128*[0|I]
    i8 = np.zeros((BS, 2, 2 * BS), np.float32)
    i8[:, 0, 0:BS] = np.eye(BS) * FSCALE
    i8[:, 1, BS : 2 * BS] = np.eye(BS) * FSCALE
    return wc.astype(BF16), w8r.astype(FP8), i8.astype(FP8)


def _elide_redundant_waits(nc):
    """Drop per-instruction semaphore waits already implied by the
    instruction's other waits (transitively, via the wait chains of the
    instructions that perform the increments). Tile's sem assignment is
    per-proc minimal but not transitively minimal across procs, and
    walrus's per-instruction sync-command budget is tiny (matmul fits
    only one wait + one update)."""
    fn = nc.m.functions[0]
    implied = {}  # sem name -> [state dict after k-th increment]
    engine_state = {}  # engine -> folded state of prior instructions' waits

    def state_of(sem, v):
        lst = implied.get(sem)
        if not lst or v <= 0:
            return {}
        return lst[min(v, len(lst)) - 1]

    def fold(dst, src):
        for s, v in src.items():
            if dst.get(s, 0) < v:
                dst[s] = v

    own_updates = {}  # engine -> {sem: count of updates emitted by this engine}
    sem_updaters = {}  # sem -> set of (engine, is_dma) that updated it
    for blk in fn.blocks:
        for inst in blk.instructions:
            si = inst.sync_info
            eng = getattr(inst, "engine", None)
            is_dma = "DMA" in type(inst).__name__
            waits = list(si.on_wait or []) if si else []
            # prune waits on this engine's own completion sem: a compute
            # engine executes serially, so all its prior updates are done
            # by the time this instruction runs. Only valid when every
            # updater of the sem so far was this engine's synchronous
            # (non-DMA) instructions.
            if eng is not None and not is_dma and waits:
                keep0 = []
                for w in waits:
                    ups = sem_updaters.get(w.ant_name)
                    if (
                        w.wait_value is not None
                        and ups is not None
                        and ups == {(eng, False)}
                        and own_updates.get(eng, {}).get(w.ant_name, 0)
                        >= w.wait_value
                    ):
                        continue
                    keep0.append(w)
                if len(keep0) != len(waits):
                    si.on_wait = keep0
                    waits = keep0
            my = dict(engine_state.get(eng, {}))
            for w in waits:
                if w.wait_value is None:
                    continue
                fold(my, {w.ant_name: w.wait_value})
                fold(my, state_of(w.ant_name, w.wait_value))
            if len(waits) > 1 and all(w.wait_value is not None for w in waits):
                keep = []
                for w in waits:
                    others = dict(engine_state.get(eng, {}))
                    for w2 in waits:
                        if w2 is w:
                            continue
                        fold(others, {w2.ant_name: w2.wait_value})
                        fold(others, state_of(w2.ant_name, w2.wait_value))
                    if others.get(w.ant_name, -1) >= w.wait_value:
                        continue
                    keep.append(w)
                if len(keep) != len(waits):
                    si.on_wait = keep
            if eng is not None:
                engine_state[eng] = my
            for u in (si.on_update or []) if si else []:
                nm = u.ant_name
                lst = implied.setdefault(nm, [])
                prev = dict(lst[-1]) if lst else {}
                fold(prev, my)
                n = u.update_value or 1
                prev[nm] = len(lst) + n
                for _ in range(int(n)):
                    lst.append(prev)
                if eng is not None:
                    eu = own_updates.setdefault(eng, {})
                    eu[nm] = eu.get(nm, 0) + int(n)
                sem_updaters.setdefault(nm, set()).add((eng, is_dma))


def _build_nc():
    nc = bass.Bass("TRN2", target_bir_lowering=False)
    x_ext = nc.declare_dram_parameter(
        "x", [ROWS_CORE, Ww, C], mybir.dt.bfloat16, isOutput=False
    )
    w_ext = nc.declare_dram_parameter(
        "wconst", [128, WC_COLS], mybir.dt.bfloat16, isOutput=False
    )
    w8_ext = nc.declare_dram_parameter(
        "wconst8", [128, 3, 2, C], mybir.dt.float8e4, isOutput=False
    )
    i8_ext = nc.declare_dram_parameter(
        "wconst8i", [BS, 2, 2 * BS], mybir.dt.float8e4, isOutput=False
    )
    out_ext = nc.declare_dram_parameter(
        "out", [TOK_CORE, C], mybir.dt.bfloat16, isOutput=True
    )

    def a_sl(k, b):
        return slice(k * NB * Ww + b * Ww, k * NB * Ww + (b + 1) * Ww)

    with tile.TileContext(nc) as tc:
        with (
            tc.tile_pool(name="const", bufs=1) as const_pool,
            tc.tile_pool(name="xin", bufs=2) as x_pool,
            tc.tile_pool(name="xs", bufs=3) as xs_pool,
            tc.tile_pool(name="gps", bufs=2, space="PSUM") as g_psum,
            tc.tile_pool(name="ops", bufs=2, space="PSUM") as o_psum,
            tc.tile_pool(name="osb", bufs=6) as out_pool,
        ):
            wc = const_pool.tile([128, WC_COLS], mybir.dt.bfloat16)
            w8 = const_pool.tile([128, 3, 2, C], mybir.dt.float8e4)
            i8 = const_pool.tile([BS, 2, 2 * BS], mybir.dt.float8e4)
            nc.sync.dma_start(wc[:, :], w_ext[:, :])
            nc.sync.dma_start(w8[:, :, :, :], w8_ext[:, :, :, :])
            nc.sync.dma_start(i8[:, :, :], i8_ext[:, :, :])

            def load_x(g):
                xt = x_pool.tile([128, RG, 2, C], mybir.dt.bfloat16, tag="xin")
                src = x_ext[g * RG : (g + 1) * RG, :, :]
                nc.gpsimd.dma_start(
                    xt[:, :, 0, :], src[:, 0:128, :].rearrange("r p c -> p r c")
                )
                nc.gpsimd.dma_start(
                    xt[0:96, :, 1, :], src[:, 128:224, :].rearrange("r p c -> p r c")
                )
                return xt

            def stage1(g, xt):
                """returns (xs8, xs8r): xs8 [96, NB, 2, TG] with slot 0 =
                fp8(Xs), slot 1 = fp8 residual (Xs - slot0); xs8r is the
                a-slots repacked 128-dense by one SBUF->SBUF DMA (channel
                permutation absorbed into the host w8 packing)."""
                xs8 = xs_pool.tile([BS, NB, 2, TG], mybir.dt.float8e4, tag="xs8")
                xs8r = xs_pool.tile([128, 6, TG], mybir.dt.float8e4, tag="xs8r")
                for b in range(NB):
                    # [96, 2 banks, 512]: rows r at (r//2, (r%2)*224)
                    pg = g_psum.tile([BS, 2, 512], mybir.dt.float32, tag="gps")
                    for r in range(RG):
                        o = (r % 2) * Ww
                        for k in range(2):
                            kp = 128 if k == 0 else 96
                            nc.tensor.matmul(
                                pg[:, r // 2, o : o + Ww],
                                lhsT=xt[0:kp, r, k, b * BS : (b + 1) * BS],
                                rhs=wc[0:kp, a_sl(k, b)],
                                start=(k == 0),
                                stop=(k == 1),
                            )
                    # last 5 blocks' a-copies ride DVE to balance ACT; DVE
                    # blocks last so the repack's DVE wait transitively
                    # implies the ACT a-copies via the pg-pool WAR chain.
                    if b < 3:
                        nc.scalar.copy(xs8[:, b, 0, :], pg[:, :, 0 : 2 * Ww])
                    else:
                        nc.vector.tensor_copy(xs8[:, b, 0, :], pg[:, :, 0 : 2 * Ww])
                    nc.vector.tensor_sub(
                        xs8[:, b, 1, :], pg[:, :, 0 : 2 * Ww], xs8[:, b, 0, :]
                    )
                # two token-halves: parallel DMA engines, and each stage-2
                # chunk's ldweights waits on only one completion sem. On the
                # gpsimd SWDGE queue: the SP queue's FIFO would serialize
                # these behind the out-DMAs and eat the pipeline slack.
                nc.gpsimd.dma_start(xs8r[:, :, 0:512], xs8[0:BS, :, 0, 0:512])
                nc.gpsimd.dma_start(xs8r[:, :, 512:TG], xs8[0:BS, :, 0, 512:TG])
                return xs8, xs8r

            def stage2(g, xs):
                xs8, xs8r = xs
                ots = []
                for j in range(TCH):
                    # one 2-bank psum tile; halves accumulate independently
                    po = o_psum.tile([128, 2, 512], mybir.dt.float32, tag="po")
                    ts = slice(j * 128, (j + 1) * 128)
                    for p in range(3):
                        lhsT = xs8r[:, 2 * p : 2 * p + 2, ts]
                        for h in range(2):
                            nc.tensor.matmul(
                                po[:, h, 0:HALF],
                                lhsT=lhsT,
                                rhs=w8[:, p, :, h * HALF : (h + 1) * HALF],
                                start=(p == 0), stop=False,
                                perf_mode=mybir.MatmulPerfMode.DoubleRow,
                                skip_group_check=True,
                            )
                    # b-residual identity, two blocks per matmul
                    for p in range(NPAIR):
                        nc.tensor.matmul(
                            po[:, p // 2, (p % 2) * 2 * BS : (p % 2 + 1) * 2 * BS],
                            lhsT=xs8[:, 2 * p : 2 * p + 2, 1, ts],
                            rhs=i8[:, :, :],
                            start=False, stop=(p % 2 == 1),
                            perf_mode=mybir.MatmulPerfMode.DoubleRow,
                            skip_group_check=True,
                        )
                    # one spare column: the 1-element gate copy absorbs
                    # ot's slot-free (DMA) wait on the ACT queue so the
                    # real copy only carries the PE wait (walrus allows
                    # one wait per Activation), without WAW overlap.
                    ot = out_pool.tile([128, C + 1], mybir.dt.bfloat16, tag="osb")
                    nc.scalar.copy(ot[0:1, C : C + 1], wc[0:1, 0:1])
                    nc.scalar.mul(ot[:, 0:C], po[:, :, 0:HALF], 1.0 / FSCALE)
                    t0 = g * TG + j * 128
                    nc.sync.dma_start(out_ext[t0 : t0 + 128, :], ot[:, 0:C])
                    ots.append(ot)
                return ots

            # software pipeline: stage2(g-2) BEFORE stage1(g) in program
            # order -- two stages of slack hide the xs8r repack DMA, and
            # the out-copies precede group g's a-copies on the ACT queue so
            # the po-pool WAR doesn't couple PE to the a-copy backlog.
            xs_hist = []
            last_ots = None
            xt = load_x(0)
            for g in range(GROUPS + 2):
                if g >= 2:
                    last_ots = stage2(g - 2, xs_hist[g - 2])
                if g < GROUPS:
                    xs_hist.append(stage1(g, xt))
                    if g + 1 < GROUPS:
                        xt = load_x(g + 1)
            # tail joins: tiny ACT writes into the last out tiles make the
            # ACT queue observe the final out-DMA completions (WAR), so the
            # kernel-tail Drain's DMA-lane waits become implied and are
            # elided (walrus allows only one wait on Drain).
            for ot in last_ots[-6:]:
                nc.scalar.copy(ot[0:1, 0:1], wc[0:1, 0:1])

    _elide_redundant_waits(nc)
    return nc


def kernel(x, block_weights, block_bias, gates, proj_w, proj_b, rescale):
    x = np.asarray(x)
    A, g = _build_amat(np.asarray(block_weights), np.asarray(gates))
    Wp = float(rescale) * np.asarray(proj_w, np.float64).T  # [c, o], no identity
    wc_host, w8_host, i8_host = _pack_weights(A, Wp)

    # shard 56 image rows per core
    x_rows = np.ascontiguousarray(x.reshape(TOK // Ww, Ww, C).astype(BF16))

    if "nc" not in _CACHE:
        _CACHE["nc"] = _build_nc()
    nc = _CACHE["nc"]

    in_maps = []
    for i in range(N_CORES):
        in_maps.append(
            {
                "x": x_rows[i * ROWS_CORE : (i + 1) * ROWS_CORE],
                "wconst": wc_host,
                "wconst8": w8_host,
                "wconst8i": i8_host,
            }
        )
    res = run_bass_kernel_spmd(
        nc,
        in_maps,
        core_ids=list(range(N_CORES)),
        trace=bool(_CACHE.get("trace", False)),
        **_CACHE.get("trace_kwargs", {}),
    )
    _CACHE["last_results"] = res
    out = np.concatenate([r["out"] for r in res.results], axis=0)
    out = out.reshape(B, Hh * Ww, C).astype(np.float32)

    # host-side constant corrections (zero for the reference inputs)
    bb = np.asarray(block_bias)
    pb = np.asarray(proj_b)
    if np.any(bb) or np.any(pb):
        P = Wp + np.eye(C)
        const = np.zeros((Hh * Ww, C), np.float64)
        if np.any(bb):
            rows = _bias_const_rows(bb, g)  # (NB, 224)
            cr = np.zeros((Ww, C), np.float64)
            for b in range(NB):
                cr[:, b * BS : (b + 1) * BS] = rows[b][:, None]
            # affects tokens with n_h == 0: tokens 0..223 of each batch image
            const[0:Ww, :] = cr @ P  # x_const goes through out = x_const @ P
        add = const[None, :, :] + float(rescale) * pb.astype(np.float64)[None, None, :]
        out = (out.astype(np.float64) + add).astype(np.float32)
    return out


# revision 33
# speedup vs baseline: 1.4256x; 1.2415x over previous
"""AFNO spectral attention kernel for 8 TRN2 NeuronCores.

Math: the reference's rfft2 -> truncate -> per-block mode mix -> irfft2
collapses to a per-block real 224x224 matrix A_b applied along the W axis
(the H-direction FFT commutes with the mode mixing and cancels), plus a
bias-driven constant on the n_h==0 spatial rows. The residual+projection
out = Xs + Xs @ (rescale*proj_w.T) runs as a mixed-precision PSUM
accumulation:
  - the heavy projection term uses fp8(e4m3) DoubleRow matmuls (K=192 per
    instruction, 2x PE rate); W' is pre-scaled by 128 so its ~0.002-scale
    entries land in fp8 normal range,
  - the identity term re-adds Xs from bf16 via scaled-identity matmuls
    (rhs = 128*I, exact in bf16) into the same PSUM bank,
  - the PSUM->SBUF copy descales by 1/128.
Only ~5.5% of the output magnitude flows through fp8, so the added error
is ~0.3% RMS.

Sharding: 100352 tokens = 8 cores x 12544 (56 complete image rows per
core, batch boundary lands exactly on the core-4 boundary). No
collectives needed.

Layout: stage-1 leaves Xs in per-block [96-partition, block, token] form
consumed directly by stage-2 (fp8 lhsT pairs two 96-channel blocks per
DoubleRow matmul), which removes the 96->128 channel repack -- the DVE
32-wide shuffle copies that dominated the previous version (86% DVE
busy). DVE now only runs the bf16->fp8 cast of Xs. Output is written
bf16 (error budget allows) and upcast on the host, halving store
traffic.
"""

import numpy as np
import ml_dtypes

import concourse.bass as bass
import concourse.mybir as mybir
import concourse.tile as tile
from concourse.bass_utils import run_bass_kernel_spmd

B, Hh, Ww, C = 2, 224, 224, 768
NB, BS, M = 8, 96, 96
NMODES = Ww // 2 + 1  # 113
N_CORES = 8
TOK = B * Hh * Ww  # 100352 total tokens
TOK_CORE = TOK // N_CORES  # 12544
ROWS_CORE = TOK_CORE // Ww  # 56 image rows per core
RG = 4  # image rows per group
GROUPS = ROWS_CORE // RG  # 14
TG = RG * Ww  # tokens per group = 896
TCH = TG // 128  # t-chunks of 128 per group = 7
A_COLS = 2 * NB * Ww  # 3584
I_COLS = BS  # identity block
WC_COLS = A_COLS + I_COLS  # 3680
NPAIR = NB // 2  # 4 block pairs for fp8 DoubleRow
HALF = NPAIR * BS  # 384 output cols per psum half
FSCALE = 128.0  # fp8 weight pre-scale (descaled on PSUM copy-out)

BF16 = ml_dtypes.bfloat16
FP8 = ml_dtypes.float8_e4m3

_CACHE = {}


def _build_amat(block_weights, gates):
    """Per-block real [224, 224] spatial-W operator."""
    g = 1.0 / (1.0 + np.exp(-gates.astype(np.float64)))
    F = np.fft.rfft(np.eye(Ww), axis=1, norm="ortho")  # (224, 113)
    A = np.zeros((NB, Ww, Ww), np.float64)
    for b in range(NB):
        T = np.zeros((NMODES, NMODES), np.complex128)
        T[:M, :M] = g[b] * block_weights[b].astype(np.float64)
        for k in range(M, NMODES):
            T[k, k] = 1.0
        A[b] = np.fft.irfft(F @ T, n=Ww, axis=1, norm="ortho")
    return A, g


def _bias_const_rows(block_bias, g):
    """Constant added to spatial rows n_h == 0, per block: (NB, 224)."""
    rows = np.zeros((NB, Ww), np.float64)
    for b in range(NB):
        z = np.zeros(NMODES, np.complex128)
        z[:M] = g[b] * block_bias[b].astype(np.float64) * (1.0 + 1.0j)
        rows[b] = np.sqrt(Hh) * np.fft.irfft(z, n=Ww, norm="ortho")
    return rows


def _pack_weights(A, Wp):
    """bf16 [128, 3680]: A chunks (k=0 rows 0:128, k=1 rows 128:224) then
    the scaled 96x96 identity; fp8 [96, 4, 2, 768]: W'*FSCALE by block
    pair."""
    wc = np.zeros((128, WC_COLS), np.float32)
    for k in range(2):
        for b in range(NB):
            blk = A[b, k * 128 : min((k + 1) * 128, Ww), :]  # (128|96, 224)
            wc[: blk.shape[0], k * NB * Ww + b * Ww : k * NB * Ww + (b + 1) * Ww] = blk
    # W' + I: the a-slot identity rides the fp8 weight diagonal (128 is
    # fp8-exact; the ~0.25-max scaled W' diagonal rounds away under the
    # ulp-16 step at 128 -- a ~0.2% RMS loss, within budget).
    WpI = Wp + np.eye(C)
    w8 = np.zeros((BS, NPAIR, 2, C), np.float32)
    for p in range(NPAIR):
        for i in range(2):
            c0 = (2 * p + i) * BS
            w8[:, p, i, :] = WpI[c0 : c0 + BS, :] * FSCALE
    # paired b-residual identity: [96, 2, 192] = 128*[I|0], 128*[0|I]
    i8 = np.zeros((BS, 2, 2 * BS), np.float32)
    i8[:, 0, 0:BS] = np.eye(BS) * FSCALE
    i8[:, 1, BS : 2 * BS] = np.eye(BS) * FSCALE
    return wc.astype(BF16), w8.astype(FP8), i8.astype(FP8)


def _elide_redundant_waits(nc):
    """Drop per-instruction semaphore waits already implied by the
    instruction's other waits (transitively, via the wait chains of the
    instructions that perform the increments). Tile's sem assignment is
    per-proc minimal but not transitively minimal across procs, and
    walrus's per-instruction sync-command budget is tiny (matmul fits
    only one wait + one update)."""
    fn = nc.m.functions[0]
    implied = {}  # sem name -> [state dict after k-th increment]
    engine_state = {}  # engine -> folded state of prior instructions' waits

    def state_of(sem, v):
        lst = implied.get(sem)
        if not lst or v <= 0:
            return {}
        return lst[min(v, len(lst)) - 1]

    def fold(dst, src):
        for s, v in src.items():
            if dst.get(s, 0) < v:
                dst[s] = v

    own_updates = {}  # engine -> {sem: count of updates emitted by this engine}
    sem_updaters = {}  # sem -> set of (engine, is_dma) that updated it
    for blk in fn.blocks:
        for inst in blk.instructions:
            si = inst.sync_info
            eng = getattr(inst, "engine", None)
            is_dma = "DMA" in type(inst).__name__
            waits = list(si.on_wait or []) if si else []
            # prune waits on this engine's own completion sem: a compute
            # engine executes serially, so all its prior updates are done
            # by the time this instruction runs. Only valid when every
            # updater of the sem so far was this engine's synchronous
            # (non-DMA) instructions.
            if eng is not None and not is_dma and waits:
                keep0 = []
                for w in waits:
                    ups = sem_updaters.get(w.ant_name)
                    if (
                        w.wait_value is not None
                        and ups is not None
                        and ups == {(eng, False)}
                        and own_updates.get(eng, {}).get(w.ant_name, 0)
                        >= w.wait_value
                    ):
                        continue
                    keep0.append(w)
                if len(keep0) != len(waits):
                    si.on_wait = keep0
                    waits = keep0
            my = dict(engine_state.get(eng, {}))
            for w in waits:
                if w.wait_value is None:
                    continue
                fold(my, {w.ant_name: w.wait_value})
                fold(my, state_of(w.ant_name, w.wait_value))
            if len(waits) > 1 and all(w.wait_value is not None for w in waits):
                keep = []
                for w in waits:
                    others = dict(engine_state.get(eng, {}))
                    for w2 in waits:
                        if w2 is w:
                            continue
                        fold(others, {w2.ant_name: w2.wait_value})
                        fold(others, state_of(w2.ant_name, w2.wait_value))
                    if others.get(w.ant_name, -1) >= w.wait_value:
                        continue
                    keep.append(w)
                if len(keep) != len(waits):
                    si.on_wait = keep
            if eng is not None:
                engine_state[eng] = my
            for u in (si.on_update or []) if si else []:
                nm = u.ant_name
                lst = implied.setdefault(nm, [])
                prev = dict(lst[-1]) if lst else {}
                fold(prev, my)
                n = u.update_value or 1
                prev[nm] = len(lst) + n
                for _ in range(int(n)):
                    lst.append(prev)
                if eng is not None:
                    eu = own_updates.setdefault(eng, {})
                    eu[nm] = eu.get(nm, 0) + int(n)
                sem_updaters.setdefault(nm, set()).add((eng, is_dma))


def _build_nc():
    nc = bass.Bass("TRN2", target_bir_lowering=False)
    x_ext = nc.declare_dram_parameter(
        "x", [ROWS_CORE, Ww, C], mybir.dt.bfloat16, isOutput=False
    )
    w_ext = nc.declare_dram_parameter(
        "wconst", [128, WC_COLS], mybir.dt.bfloat16, isOutput=False
    )
    w8_ext = nc.declare_dram_parameter(
        "wconst8", [BS, NPAIR, 2, C], mybir.dt.float8e4, isOutput=False
    )
    i8_ext = nc.declare_dram_parameter(
        "wconst8i", [BS, 2, 2 * BS], mybir.dt.float8e4, isOutput=False
    )
    out_ext = nc.declare_dram_parameter(
        "out", [TOK_CORE, C], mybir.dt.bfloat16, isOutput=True
    )

    def a_sl(k, b):
        return slice(k * NB * Ww + b * Ww, k * NB * Ww + (b + 1) * Ww)

    with tile.TileContext(nc) as tc:
        with (
            tc.tile_pool(name="const", bufs=1) as const_pool,
            tc.tile_pool(name="xin", bufs=2) as x_pool,
            tc.tile_pool(name="xs", bufs=3) as xs_pool,
            tc.tile_pool(name="gps", bufs=2, space="PSUM") as g_psum,
            tc.tile_pool(name="ops", bufs=2, space="PSUM") as o_psum,
            tc.tile_pool(name="osb", bufs=6) as out_pool,
        ):
            wc = const_pool.tile([128, WC_COLS], mybir.dt.bfloat16)
            w8 = const_pool.tile([BS, NPAIR, 2, C], mybir.dt.float8e4)
            i8 = const_pool.tile([BS, 2, 2 * BS], mybir.dt.float8e4)
            nc.sync.dma_start(wc[:, :], w_ext[:, :])
            nc.sync.dma_start(w8[:, :, :, :], w8_ext[:, :, :, :])
            nc.sync.dma_start(i8[:, :, :], i8_ext[:, :, :])

            def load_x(g):
                xt = x_pool.tile([128, RG, 2, C], mybir.dt.bfloat16, tag="xin")
                src = x_ext[g * RG : (g + 1) * RG, :, :]
                nc.gpsimd.dma_start(
                    xt[:, :, 0, :], src[:, 0:128, :].rearrange("r p c -> p r c")
                )
                nc.gpsimd.dma_start(
                    xt[0:96, :, 1, :], src[:, 128:224, :].rearrange("r p c -> p r c")
                )
                return xt

            def stage1(g, xt):
                """returns xs8 [96, NB, 2, TG]: slot 0 = fp8(Xs), slot 1 =
                fp8 residual (Xs - slot0); identity re-adds slot0+slot1."""
                xs8 = xs_pool.tile([BS, NB, 2, TG], mybir.dt.float8e4, tag="xs8")
                for b in range(NB):
                    # [96, 2 banks, 512]: rows r at (r//2, (r%2)*224)
                    pg = g_psum.tile([BS, 2, 512], mybir.dt.float32, tag="gps")
                    for r in range(RG):
                        o = (r % 2) * Ww
                        for k in range(2):
                            kp = 128 if k == 0 else 96
                            nc.tensor.matmul(
                                pg[:, r // 2, o : o + Ww],
                                lhsT=xt[0:kp, r, k, b * BS : (b + 1) * BS],
                                rhs=wc[0:kp, a_sl(k, b)],
                                start=(k == 0),
                                stop=(k == 1),
                            )
                    nc.scalar.copy(xs8[:, b, 0, :], pg[:, :, 0 : 2 * Ww])
                    nc.vector.tensor_sub(
                        xs8[:, b, 1, :], pg[:, :, 0 : 2 * Ww], xs8[:, b, 0, :]
                    )
                return xs8

            def stage2(g, xs8):
                ots = []
                for j in range(TCH):
                    # one 2-bank psum tile; halves accumulate independently
                    po = o_psum.tile([128, 2, 512], mybir.dt.float32, tag="po")
                    ts = slice(j * 128, (j + 1) * 128)
                    for p in range(NPAIR):
                        lhsT = xs8[:, 2 * p : 2 * p + 2, 0, ts]
                        for h in range(2):
                            nc.tensor.matmul(
                                po[:, h, 0:HALF],
                                lhsT=lhsT,
                                rhs=w8[:, p, :, h * HALF : (h + 1) * HALF],
                                start=(p == 0), stop=False,
                                perf_mode=mybir.MatmulPerfMode.DoubleRow,
                                skip_group_check=True,
                            )
                    # b-residual identity, two blocks per matmul
                    for p in range(NPAIR):
                        nc.tensor.matmul(
                            po[:, p // 2, (p % 2) * 2 * BS : (p % 2 + 1) * 2 * BS],
                            lhsT=xs8[:, 2 * p : 2 * p + 2, 1, ts],
                            rhs=i8[:, :, :],
                            start=False, stop=(p % 2 == 1),
                            perf_mode=mybir.MatmulPerfMode.DoubleRow,
                            skip_group_check=True,
                        )
                    # one spare column: the 1-element gate copy absorbs
                    # ot's slot-free (DMA) wait on the ACT queue so the
                    # real copy only carries the PE wait (walrus allows
                    # one wait per Activation), without WAW overlap.
                    ot = out_pool.tile([128, C + 1], mybir.dt.bfloat16, tag="osb")
                    nc.scalar.copy(ot[0:1, C : C + 1], wc[0:1, 0:1])
                    nc.scalar.mul(ot[:, 0:C], po[:, :, 0:HALF], 1.0 / FSCALE)
                    t0 = g * TG + j * 128
                    nc.sync.dma_start(out_ext[t0 : t0 + 128, :], ot[:, 0:C])
                    ots.append(ot)
                return ots

            # software pipeline: stage2(g-2) BEFORE stage1(g) in program
            # order -- two stages of slack hide the xs8r repack DMA, and
            # the out-copies precede group g's a-copies on the ACT queue so
            # the po-pool WAR doesn't couple PE to the a-copy backlog.
            xs_hist = []
            last_ots = None
            xt = load_x(0)
            for g in range(GROUPS + 2):
                if g >= 2:
                    last_ots = stage2(g - 2, xs_hist[g - 2])
                if g < GROUPS:
                    xs_hist.append(stage1(g, xt))
                    if g + 1 < GROUPS:
                        xt = load_x(g + 1)
            # tail joins: tiny ACT writes into the last out tiles make the
            # ACT queue observe the final out-DMA completions (WAR), so the
            # kernel-tail Drain's DMA-lane waits become implied and are
            # elided (walrus allows only one wait on Drain).
            for ot in last_ots[-6:]:
                nc.scalar.copy(ot[0:1, 0:1], wc[0:1, 0:1])

    _elide_redundant_waits(nc)
    return nc


def kernel(x, block_weights, block_bias, gates, proj_w, proj_b, rescale):
    x = np.asarray(x)
    A, g = _build_amat(np.asarray(block_weights), np.asarray(gates))
    Wp = float(rescale) * np.asarray(proj_w, np.float64).T  # [c, o], no identity
    wc_host, w8_host, i8_host = _pack_weights(A, Wp)

    # shard 56 image rows per core
    x_rows = np.ascontiguousarray(x.reshape(TOK // Ww, Ww, C).astype(BF16))

    if "nc" not in _CACHE:
        _CACHE["nc"] = _build_nc()
    nc = _CACHE["nc"]

    in_maps = []
    for i in range(N_CORES):
        in_maps.append(
            {
                "x": x_rows[i * ROWS_CORE : (i + 1) * ROWS_CORE],
                "wconst": wc_host,
                "wconst8": w8_host,
                "wconst8i": i8_host,
            }
        )
    res = run_bass_kernel_spmd(
        nc,
        in_maps,
        core_ids=list(range(N_CORES)),
        trace=bool(_CACHE.get("trace", False)),
        **_CACHE.get("trace_kwargs", {}),
    )
    _CACHE["last_results"] = res
    out = np.concatenate([r["out"] for r in res.results], axis=0)
    out = out.reshape(B, Hh * Ww, C).astype(np.float32)

    # host-side constant corrections (zero for the reference inputs)
    bb = np.asarray(block_bias)
    pb = np.asarray(proj_b)
    if np.any(bb) or np.any(pb):
        P = Wp + np.eye(C)
        const = np.zeros((Hh * Ww, C), np.float64)
        if np.any(bb):
            rows = _bias_const_rows(bb, g)  # (NB, 224)
            cr = np.zeros((Ww, C), np.float64)
            for b in range(NB):
                cr[:, b * BS : (b + 1) * BS] = rows[b][:, None]
            # affects tokens with n_h == 0: tokens 0..223 of each batch image
            const[0:Ww, :] = cr @ P  # x_const goes through out = x_const @ P
        add = const[None, :, :] + float(rescale) * pb.astype(np.float64)[None, None, :]
        out = (out.astype(np.float64) + add).astype(np.float32)
    return out
